# revision 29
# baseline (speedup 1.0000x reference)
"""MLA (multi-head latent attention) Bass kernel for Trainium2, 8 NeuronCores.

Sharding: core i handles batch b = i // 2 and head-group g = i % 2
(8 of the 16 heads).  Each core computes a partial output (its heads'
contribution through out_proj); the host sums the two partials per batch
and adds a constant row (b_kvu_v @ w_o + b_o), which is exact because
softmax rows sum to 1 so the V-bias passes through attention additively.

All matmul operands are bf16 (1 cycle/row on the PE regardless of
output width); accumulation stays f32 in PSUM.  No PE transposes: both
x -> xT and ctx -> ctxT go through the DMA XBAR (dma_start_transpose,
2-byte dtypes) after an f32->bf16 rounding copy on GpSimd/DVE.

Pipeline (single TileContext; emission interleaved so attention starts
~20us in and out_proj overlaps the second attention half):
  piece(p), p=0..3 (512 tokens each):
    x chunks DMA'd, rounded to bf16 on Pool, DMA-transposed into
    xT [128, 8 d-chunks, S]; latents kv_latT [128,S], q_latT{0,1}
    [128,S] = W^T xT (+bias, DVE); KT/QT [128, 4 chunks * S] and
    V [128, NT*520] (64 cols/head + ones col for the softmax denom).
  attention(j, hp) per s-half j and head pair hp, heads sequential:
    scoresT [128 keys, 1024 queries] per key-chunk k via PE (64-row
    operands, disjoint groups per head); exp on ScalarE (scale=1/8,
    bf16 out); causal diagonal via affine_select on Pool; PV re-uses
    exp tiles as stationary: ctx_psum[s-chunk] [128 queries, 65]
    accumulates over k with the ones column giving the denominator.
    Retire: strided reciprocal [128,8] + 8 per-partition scalar
    multiplies (DVE) into a token-major bf16 pair tile, then one DMA
    transpose per (j,hp) into ctxT [128, 4 chunks * S].
  out_proj per 128-token chunk: 4x128-contraction accumulate into
  [128,512] PSUM halves, copies split DVE/Pool, DMA out.
"""

import numpy as np

import concourse.bass as bass
import concourse.bacc as bacc
import concourse.mybir as mybir
import concourse.tile as tile

DIM = 1024
NUM_HEADS = 16
HEAD_DIM = 64
LAT = 128
QR = 256
B = 4
NCORES = 8
ND = DIM // 128       # 8 d-chunks
NHL = 8               # heads per core
F32 = mybir.dt.float32
BF16 = mybir.dt.bfloat16
AF = mybir.ActivationFunctionType


def _pieces(total, w=512):
    return [(o, min(w, total - o)) for o in range(0, total, w)]


def build_mla(S=2048, mmdt=BF16):
    """Build the per-core Bass program (same SPMD program on all 8 cores)."""
    assert S % 512 == 0
    SH = S // 2           # s-half width
    NT = S // 128         # number of 128-token chunks
    NP = S // 512         # number of 512-token pieces

    nc = bacc.Bacc()

    x_d = nc.declare_dram_parameter("x", [S, DIM], F32, isOutput=False)
    w_kvc_d = nc.declare_dram_parameter("w_kvc", [DIM, LAT], F32, isOutput=False)
    w_qc_d = nc.declare_dram_parameter("w_qc", [DIM, QR], F32, isOutput=False)
    w_kvu_k_d = nc.declare_dram_parameter("w_kvu_k", [LAT, 512], F32, isOutput=False)
    w_kvu_v_d = nc.declare_dram_parameter("w_kvu_v", [LAT, 512], F32, isOutput=False)
    w_qu_d = nc.declare_dram_parameter("w_qu", [QR, 512], F32, isOutput=False)
    w_o_d = nc.declare_dram_parameter("w_o", [512, DIM], F32, isOutput=False)
    b_all_d = nc.declare_dram_parameter("b_all", [128, 11], F32, isOutput=False)
    out_d = nc.declare_dram_parameter("out", [S, DIM], F32, isOutput=True)

    with tile.TileContext(nc) as tc:
        with (
            tc.tile_pool(name="wts", bufs=1) as wts,
            tc.tile_pool(name="big", bufs=1) as big,
            tc.tile_pool(name="stg", bufs=2) as stg,
            tc.tile_pool(name="xfp", bufs=2) as xfp,
            tc.tile_pool(name="xbp", bufs=2) as xbp,
            tc.tile_pool(name="attn", bufs=1) as attn,
            tc.tile_pool(name="cpp", bufs=2) as cpp,
            tc.tile_pool(name="obp", bufs=3) as obp,
            tc.tile_pool(name="scps", bufs=1, space="PSUM") as scps,
            tc.tile_pool(name="ctxps", bufs=1, space="PSUM") as ctxps,
        ):
            # ---- persistent products -----------------------------------
            xT = big.tile([128, ND * S], mmdt, name="xT")
            xT_v = xT[:].rearrange("p (d t) -> p d t", d=ND)
            kv_latT = big.tile([128, S], mmdt, name="kv_latT")
            q_latT0 = big.tile([128, S], mmdt, name="q_latT0")
            q_latT1 = big.tile([128, S], mmdt, name="q_latT1")
            KT = big.tile([128, 4 * S], mmdt, name="KT")
            QT = big.tile([128, 4 * S], mmdt, name="QT")
            V = big.tile([128, NT * 520], mmdt, name="V")
            v_view = V[:].rearrange("p (k h c) -> p k h c", h=NHL, c=65)
            ctxT = big.tile([128, 4 * S], mmdt, name="ctxT")
            ctxT_v = ctxT[:].rearrange("p (c t) -> p c t", c=4)

            # ones columns of V (col 64 of each 65-wide head block)
            nc.gpsimd.memset(v_view[:, :, :, 64:65], 1.0)

            # ---- weights into SBUF (staged fp32 DMA, rounded to bf16) --
            def load_rounded(dst_ap, src_ap, shape):
                st = stg.tile([128, 1024], F32, tag="stage")
                sap = st[:shape[0], :shape[1]]
                nc.sync.dma_start(out=sap, in_=src_ap)
                nc.vector.tensor_copy(dst_ap, sap)

            w_kvc_sb = wts.tile([128, DIM], mmdt, name="w_kvc_sb")
            w_qc_sb = wts.tile([128, ND * QR], mmdt, name="w_qc_sb")
            w_kvu_k_sb = wts.tile([128, 512], mmdt, name="w_kvu_k_sb")
            w_kvu_v_sb = wts.tile([128, 512], mmdt, name="w_kvu_v_sb")
            w_qu_sb = wts.tile([128, 1024], mmdt, name="w_qu_sb")
            w_o_sb = wts.tile([128, 4 * DIM], mmdt, name="w_o_sb")
            b_all_sb = wts.tile([128, 11], F32, name="b_all_sb")
            b_kvc_sb = b_all_sb[:, 0:1]
            b_qc_sb = b_all_sb[:, 1:3]
            b_qu_sb = b_all_sb[:, 3:7]
            b_kvu_k_sb = b_all_sb[:, 7:11]

            def load_rounded3(dst_ap, src3_ap, nchunks, w):
                """One DMA of [128, nchunks, w] row-chunked DRAM weights."""
                st = stg.tile([128, 1024], F32, tag="stage")
                sap = st[:, :nchunks * w].rearrange("p (c q) -> p c q", c=nchunks)
                nc.sync.dma_start(out=sap, in_=src3_ap)
                nc.vector.tensor_copy(dst_ap, st[:, :nchunks * w])

            def wload_latent():
                nc.sync.dma_start(out=b_all_sb[:], in_=b_all_d[:, :])
                load_rounded3(w_kvc_sb[:],
                              w_kvc_d[:, :].rearrange("(c p) q -> p c q", p=128),
                              ND, 128)
                for g in range(2):
                    load_rounded3(
                        w_qc_sb[:, 1024 * g:1024 * g + 1024],
                        w_qc_d[512 * g:512 * g + 512, :].rearrange(
                            "(c p) q -> p c q", p=128),
                        4, QR)

            def wload_up():
                load_rounded3(w_qu_sb[:],
                              w_qu_d[:, :].rearrange("(c p) q -> p c q", p=128),
                              2, 512)
                load_rounded(w_kvu_k_sb[:], w_kvu_k_d[:, :], (128, 512))
                load_rounded(w_kvu_v_sb[:], w_kvu_v_d[:, :], (128, 512))


            def wload_o():
                for cc in range(4):
                    load_rounded(w_o_sb[:, DIM * cc:DIM * cc + DIM],
                                 w_o_d[128 * cc:128 * cc + 128, :], (128, DIM))

            # ---- emission helpers --------------------------------------
            def x_chunk(q):
                xf = xfp.tile([128, DIM], F32, tag="xf", bufs=2)
                nc.sync.dma_start(
                    out=xf[:], in_=x_d[128 * q:128 * q + 128, :])
                xb = xbp.tile([128, DIM], mmdt, tag="xb", bufs=3)
                nc.gpsimd.tensor_copy(xb[:], xf[:])
                nc.sync.dma_start_transpose(
                    xT_v[:, :, 128 * q:128 * q + 128], xb[:])

            def x_stage(p):
                """Load, round, and DMA-transpose x tokens [512p, 512p+512)."""
                for q in range(4 * p, 4 * p + 4):
                    xf = xfp.tile([128, DIM], F32, tag="xf", bufs=2)
                    nc.sync.dma_start(
                        out=xf[:], in_=x_d[128 * q:128 * q + 128, :])
                    xb = xbp.tile([128, DIM], mmdt, tag="xb", bufs=3)
                    nc.gpsimd.tensor_copy(xb[:], xf[:])
                    nc.sync.dma_start_transpose(
                        xT_v[:, :, 128 * q:128 * q + 128], xb[:])

            def piece(pj, p, to_bg=False):
                """All projections for tokens [512p, 512p+512).

                With to_bg=True the sub-steps are queued on `bg` and drained
                one per attention slot, so they fill engine-idle time instead
                of preempting the next unit's QK matmuls.
                """
                o = 512 * p
                items = []

                def _lat(w_sb, coloff, dst, b_ap):
                    def emit():
                        acc = pj.tile([128, 512], F32, tag="pj", bufs=2,
                                      name=f"pj_{p}_{coloff}")
                        for dc in range(ND):
                            nc.tensor.matmul(
                                acc[:], w_sb[:, QR * dc + coloff:QR * dc + coloff + 128]
                                if w_sb is w_qc_sb else
                                w_sb[:, 128 * dc:128 * dc + 128],
                                xT_v[:, dc, o:o + 512],
                                start=(dc == 0), stop=(dc == ND - 1))
                        nc.vector.tensor_scalar_add(dst[:, o:o + 512], acc[:], b_ap)
                    return emit

                items.append(_lat(w_kvc_sb, 0, kv_latT, b_kvc_sb))
                items.append(_lat(w_qc_sb, 0, q_latT0, b_qc_sb[:, 0:1]))
                items.append(_lat(w_qc_sb, 128, q_latT1, b_qc_sb[:, 1:2]))

                def _qt(c):
                    def emit():
                        qp2 = pj.tile([128, 512], F32, tag="pj", bufs=2,
                                      name=f"pjq_{p}_{c}")
                        nc.tensor.matmul(
                            qp2[:], w_qu_sb[:, 128 * c:128 * c + 128],
                            q_latT0[:, o:o + 512], start=True, stop=False)
                        nc.tensor.matmul(
                            qp2[:], w_qu_sb[:, 512 + 128 * c:512 + 128 * c + 128],
                            q_latT1[:, o:o + 512], start=False, stop=True)
                        nc.vector.tensor_scalar_add(
                            QT[:, c * S + o:c * S + o + 512], qp2[:],
                            b_qu_sb[:, c:c + 1])
                        kp = pj.tile([128, 512], F32, tag="pj", bufs=2,
                                     name=f"pjk_{p}_{c}")
                        nc.tensor.matmul(
                            kp[:], w_kvu_k_sb[:, 128 * c:128 * c + 128],
                            kv_latT[:, o:o + 512], start=True, stop=True)
                        nc.vector.tensor_scalar_add(
                            KT[:, c * S + o:c * S + o + 512], kp[:],
                            b_kvu_k_sb[:, c:c + 1])
                    return emit

                for c in range(4):
                    items.append(_qt(c))

                def _v(q):
                    def emit():
                        vp = pj.tile([128, 512], F32, tag="pj", bufs=2,
                                     name=f"pjv_{q}")
                        nc.tensor.matmul(vp[:], kv_latT[:, 128 * q:128 * q + 128],
                                         w_kvu_v_sb[:], start=True, stop=True)
                        nc.vector.tensor_copy(
                            v_view[:, q, :, 0:64],
                            vp[:].rearrange("p (h c) -> p h c", c=64))
                    return emit

                for q in range(4 * p, 4 * p + 4):
                    items.append(_v(q))
                if to_bg:
                    bg.extend(items)
                else:
                    for it in items:
                        it()

            pending = []  # deferred closures, drained into later QK/exp slots
            bg = []       # background closures (pieces, out_proj), 1 per slot

            def drain(n):
                for _ in range(min(n, len(pending))):
                    pending.pop(0)()

            def drain_bg(n):
                for _ in range(min(n, len(bg))):
                    bg.pop(0)()

            def attn_unit(hp, s0, SW=512, last=False):
                """Attention for queries [s0, s0+SW), both heads of pair hp.

                One merged exp per key-chunk covers both heads ([128, 2, fd]
                strided AP over a shared scores tile).  PV accumulation
                groups are emitted contiguously per (head, s-chunk) -- PSUM
                banks support only one open group at a time -- lagged into
                the QK/exp slots; trailing groups, the retires, and the
                ctxT transposes are deferred into the NEXT unit's slots via
                `pending` (the 4-deep PE wait queue would otherwise block
                the next unit's QK behind not-yet-ready PV matmuls).
                """
                base = s0 // 128
                nch = SW // 128
                kmax = base + nch
                cp = cpp.tile([128, 512], mmdt, tag="cp", name=f"cp_{hp}_{s0}")
                ctxs = [ctxps.tile([128, 512], F32, tag="ctx", bufs=2,
                                   name=f"ctx_{hp}_{s0}_{i}")
                        for i in range(2)]
                exs = []

                def pv_group(h2, c):
                    h = 2 * hp + h2
                    ctx = ctxs[h2]
                    klast = base + c

                    def emit():
                        for k in range(klast + 1):
                            rel = max(s0, 128 * k) - s0
                            cs = 512 * h2 + 128 * c - rel
                            nc.tensor.matmul(
                                ctx[:, 128 * c:128 * c + 65],
                                exs[k][:, cs:cs + 128],
                                V[:, 520 * k + 65 * h:520 * k + 65 * h + 65],
                                start=(k == 0), stop=(k == klast))
                        if last:
                            rec = attn.tile([128, 1], F32, tag="rec1", bufs=4,
                                            name=f"rec1_{hp}_{s0}_{h2}_{c}")
                            nc.vector.reciprocal(
                                rec[:], ctx[:, 128 * c + 64:128 * c + 65])
                            nc.vector.tensor_scalar_mul(
                                cp[:, 128 * c + 64 * h2:128 * c + 64 * h2 + 64],
                                ctx[:, 128 * c:128 * c + 64], rec[:, 0:1])
                            if h2 == 1:
                                nc.sync.dma_start_transpose(
                                    ctxT_v[:, hp, s0 + 128 * c:s0 + 128 * c + 128]
                                    .rearrange("p (b t) -> p b t", t=128),
                                    cp[:, 128 * c:128 * c + 128])
                    return emit

                def retire(h2):
                    ctx = ctxs[h2]

                    def emit():
                        rec = attn.tile([128, 4], F32, tag="rec", bufs=4,
                                        name=f"rec_{hp}_{s0}_{h2}")
                        nc.vector.reciprocal(
                            rec[:, :nch],
                            ctx[:].rearrange("p (c u) -> p c u", u=128)[:, :nch, 64])
                        for c in range(nch):
                            nc.vector.tensor_scalar_mul(
                                cp[:, 128 * c + 64 * h2:128 * c + 64 * h2 + 64],
                                ctx[:, 128 * c:128 * c + 64], rec[:, c:c + 1])
                    return emit

                def tp():
                    def emit():
                        nc.sync.dma_start_transpose(
                            ctxT_v[:, hp, s0:s0 + SW].rearrange(
                                "p (b t) -> p b t", t=128),
                            cp[:, :SW])
                    return emit

                lag = 2 if s0 == 0 else 0
                inslot = nch if last else max(1, nch - 2)
                emitted = 0
                for k in range(kmax):
                    t0 = 128 * k
                    ss = max(s0, t0)
                    fd = s0 + SW - ss
                    sc = scps.tile([128, 1024], F32, tag="sc", bufs=2,
                                   name=f"sc_{hp}_{s0}_{k}")
                    for h2 in range(2):
                        po = 64 * h2
                        nc.tensor.matmul(
                            sc[:, 512 * h2:512 * h2 + fd],
                            KT[po:po + 64, hp * S + t0:hp * S + t0 + 128],
                            QT[po:po + 64, hp * S + ss:hp * S + ss + fd],
                            start=True, stop=True)
                    ex = attn.tile([128, 1024], mmdt, tag="ex", bufs=21,
                                   name=f"ex_{hp}_{s0}_{k}")
                    exs.append(ex)
                    sc3 = sc[:].rearrange("p (g q) -> p g q", g=2)[:, :, :fd]
                    ex3 = ex[:].rearrange("p (g q) -> p g q", g=2)[:, :, :fd]
                    nc.scalar.activation(ex3, sc3, AF.Exp, scale=0.125)
                    if t0 >= s0:
                        nc.gpsimd.affine_select(
                            out=ex[:].rearrange("p (g q) -> p g q", g=2)[:, :, 0:128],
                            in_=ex[:].rearrange("p (g q) -> p g q", g=2)[:, :, 0:128],
                            pattern=[[0, 2], [1, 128]],
                            compare_op=mybir.AluOpType.is_ge,
                            fill=0.0, base=0, channel_multiplier=-1)
                    drain(2)
                    drain_bg(1)
                    c = k - base - lag
                    if 0 <= c < inslot:
                        pv_group(0, c)()
                        pv_group(1, c)()
                        emitted = c + 1
                for c in range(emitted, nch):
                    if last:
                        pv_group(0, c)()
                        pv_group(1, c)()
                    else:
                        pending.append(pv_group(0, c))
                        pending.append(pv_group(1, c))
                if not last:
                    pending.append(retire(0))
                    pending.append(retire(1))
                    pending.append(tp())

            def out_chunk(ops, si, to_bg=False):
                """out_proj for tokens [128si, 128si+128)."""
                ob = obp.tile([128, DIM], F32, tag="ob", name=f"ob_{si}")

                def _half(u):
                    def emit():
                        op = ops.tile([128, 512], F32, tag="op", bufs=2,
                                      name=f"op_{si}_{u}")
                        for cc in range(4):
                            nc.tensor.matmul(
                                op[:],
                                ctxT_v[:, cc, 128 * si:128 * si + 128],
                                w_o_sb[:, DIM * cc + 512 * u:DIM * cc + 512 * u + 512],
                                start=(cc == 0), stop=(cc == 3))
                        nc.vector.tensor_copy(ob[:, 512 * u:512 * u + 512], op[:])
                        if u == 1:
                            nc.sync.dma_start(
                                out=out_d[128 * si:128 * si + 128, :], in_=ob[:])
                    return emit

                if to_bg:
                    bg.append(_half(0))
                    bg.append(_half(1))
                else:
                    _half(0)()
                    _half(1)()

            # ---- emission schedule -------------------------------------
            with tc.tile_pool(name="pjps", bufs=1, space="PSUM") as pj:
                x_stage(0)
                wload_latent()
                # interleave the second x piece with the up-projection weight
                # loads: w_qu gates piece-0's QT (the first attention pass),
                # the x chunks gate piece 1 (the second)
                load_rounded3(w_qu_sb[:],
                              w_qu_d[:, :].rearrange("(c p) q -> p c q", p=128),
                              2, 512)
                x_chunk(4)
                load_rounded(w_kvu_k_sb[:], w_kvu_k_d[:, :], (128, 512))
                x_chunk(5)
                load_rounded(w_kvu_v_sb[:], w_kvu_v_d[:, :], (128, 512))
                x_chunk(6)
                x_chunk(7)
                piece(pj, 0)
                # piece-0's V chunks come back off the critical path: queue
                # behind the first attention units
                for hp in range(4):
                    attn_unit(hp, 0)
                piece(pj, 1)
                x_stage(2)
                x_stage(3)
                piece(pj, 2, to_bg=True)
                piece(pj, 3, to_bg=True)
                for hp in range(4):
                    attn_unit(hp, 512)
                wload_o()
            with tc.tile_pool(name="ops", bufs=1, space="PSUM") as ops:
                for hp in range(4):
                    attn_unit(hp, 1024)
                    out_chunk(ops, 2 * hp, to_bg=True)
                    out_chunk(ops, 2 * hp + 1, to_bg=True)
                for hp in range(3):
                    attn_unit(hp, 1536)
                    out_chunk(ops, 8 + hp, to_bg=True)
                out_chunk(ops, 11, to_bg=True)
                attn_unit(3, 1536, SW=256)
                attn_unit(3, 1792, SW=256, last=True)
                drain(len(pending))
                drain_bg(len(bg))
                for si in range(12, NT):
                    out_chunk(ops, si)

    nc.finalize()
    return nc


def shard_inputs(inputs, S=2048):
    """Build the 8 per-core input maps from full inputs."""
    f = lambda a: np.ascontiguousarray(np.asarray(a, dtype=np.float32))
    x = f(inputs["x"])
    w_kvc, b_kvc = f(inputs["w_kvc"]), f(inputs["b_kvc"])
    w_kvu, b_kvu = f(inputs["w_kvu"]), f(inputs["b_kvu"])
    w_qc, b_qc = f(inputs["w_qc"]), f(inputs["b_qc"])
    w_qu, b_qu = f(inputs["w_qu"]), f(inputs["b_qu"])
    w_o = f(inputs["w_o"])
    in_maps = []
    for core in range(NCORES):
        b = core // 2
        g = core % 2
        cs = slice(512 * g, 512 * g + 512)
        in_maps.append({
            "x": x[b],
            "w_kvc": w_kvc,
            "w_qc": w_qc,
            "w_kvu_k": np.ascontiguousarray(w_kvu[:, 512 * g:512 * g + 512]),
            "w_kvu_v": np.ascontiguousarray(w_kvu[:, 1024 + 512 * g:1024 + 512 * g + 512]),
            "w_qu": np.ascontiguousarray(w_qu[:, cs]),
            "w_o": np.ascontiguousarray(w_o[cs, :]),
            "b_all": np.ascontiguousarray(np.concatenate([
                b_kvc.reshape(128, 1),
                b_qc.reshape(2, 128).T,
                b_qu[cs].reshape(4, 128).T,
                b_kvu[cs].reshape(4, 128).T,
            ], axis=1)),
        })
    return in_maps


def gather_out(results, inputs, S=2048):
    """Sum the two per-batch partials and add the constant bias row."""
    f = lambda a: np.asarray(a, dtype=np.float32)
    b_v = f(inputs["b_kvu"])[DIM:]
    const_row = b_v @ f(inputs["w_o"]) + f(inputs["b_o"])
    out = np.empty((B, S, DIM), dtype=np.float32)
    for b in range(B):
        out[b] = results[2 * b]["out"] + results[2 * b + 1]["out"] + const_row
    return out


def kernel(**inputs) -> np.ndarray:
    from concourse.bass_utils import run_bass_kernel_spmd

    x = np.asarray(inputs["x"])
    S = x.shape[1]
    nc = build_mla(S=S)
    in_maps = shard_inputs(inputs, S=S)
    res = run_bass_kernel_spmd(nc, in_maps, list(range(NCORES))).results
    return gather_out(res, inputs, S=S)


# revision 32
# speedup vs baseline: 1.0248x; 1.0248x over previous
"""MLA (multi-head latent attention) Bass kernel for Trainium2, 8 NeuronCores.

Sharding: core i handles batch b = i // 2 and head-group g = i % 2
(8 of the 16 heads).  Each core computes a partial output (its heads'
contribution through out_proj); the host sums the two partials per batch
and adds a constant row (b_kvu_v @ w_o + b_o), which is exact because
softmax rows sum to 1 so the V-bias passes through attention additively.

All matmul operands are bf16 (1 cycle/row on the PE regardless of
output width); accumulation stays f32 in PSUM.  No PE transposes: both
x -> xT and ctx -> ctxT go through the DMA XBAR (dma_start_transpose,
2-byte dtypes) after an f32->bf16 rounding copy on GpSimd/DVE.

Pipeline (single TileContext; emission interleaved so attention starts
~20us in and out_proj overlaps the second attention half):
  piece(p), p=0..3 (512 tokens each):
    x chunks DMA'd, rounded to bf16 on Pool, DMA-transposed into
    xT [128, 8 d-chunks, S]; latents kv_latT [128,S], q_latT{0,1}
    [128,S] = W^T xT (+bias, DVE); KT/QT [128, 4 chunks * S] and
    V [128, NT*520] (64 cols/head + ones col for the softmax denom).
  attention(j, hp) per s-half j and head pair hp, heads sequential:
    scoresT [128 keys, 1024 queries] per key-chunk k via PE (64-row
    operands, disjoint groups per head); exp on ScalarE (scale=1/8,
    bf16 out); causal diagonal via affine_select on Pool; PV re-uses
    exp tiles as stationary: ctx_psum[s-chunk] [128 queries, 65]
    accumulates over k with the ones column giving the denominator.
    Retire: strided reciprocal [128,8] + 8 per-partition scalar
    multiplies (DVE) into a token-major bf16 pair tile, then one DMA
    transpose per (j,hp) into ctxT [128, 4 chunks * S].
  out_proj per 128-token chunk: 4x128-contraction accumulate into
  [128,512] PSUM halves, copies split DVE/Pool, DMA out.
"""

import numpy as np

import concourse.bass as bass
import concourse.bacc as bacc
import concourse.mybir as mybir
import concourse.tile as tile

DIM = 1024
NUM_HEADS = 16
HEAD_DIM = 64
LAT = 128
QR = 256
B = 4
NCORES = 8
ND = DIM // 128       # 8 d-chunks
NHL = 8               # heads per core
F32 = mybir.dt.float32
BF16 = mybir.dt.bfloat16
AF = mybir.ActivationFunctionType


def _pieces(total, w=512):
    return [(o, min(w, total - o)) for o in range(0, total, w)]


def build_mla(S=2048, mmdt=BF16):
    """Build the per-core Bass program (same SPMD program on all 8 cores)."""
    assert S % 512 == 0
    SH = S // 2           # s-half width
    NT = S // 128         # number of 128-token chunks
    NP = S // 512         # number of 512-token pieces

    nc = bacc.Bacc()

    x_d = nc.declare_dram_parameter("x", [S, DIM], F32, isOutput=False)
    w_kvc_d = nc.declare_dram_parameter("w_kvc", [DIM, LAT], F32, isOutput=False)
    w_qc_d = nc.declare_dram_parameter("w_qc", [DIM, QR], F32, isOutput=False)
    w_kvu_k_d = nc.declare_dram_parameter("w_kvu_k", [LAT, 512], F32, isOutput=False)
    w_kvu_v_d = nc.declare_dram_parameter("w_kvu_v", [LAT, 512], F32, isOutput=False)
    w_qu_d = nc.declare_dram_parameter("w_qu", [QR, 512], F32, isOutput=False)
    w_o_d = nc.declare_dram_parameter("w_o", [512, DIM], F32, isOutput=False)
    b_all_d = nc.declare_dram_parameter("b_all", [128, 11], F32, isOutput=False)
    out_d = nc.declare_dram_parameter("out", [S, DIM], F32, isOutput=True)

    with tile.TileContext(nc) as tc:
        with (
            tc.tile_pool(name="wts", bufs=1) as wts,
            tc.tile_pool(name="big", bufs=1) as big,
            tc.tile_pool(name="stg", bufs=2) as stg,
            tc.tile_pool(name="xfp", bufs=2) as xfp,
            tc.tile_pool(name="xbp", bufs=2) as xbp,
            tc.tile_pool(name="attn", bufs=1) as attn,
            tc.tile_pool(name="cpp", bufs=2) as cpp,
            tc.tile_pool(name="obp", bufs=3) as obp,
            tc.tile_pool(name="scps", bufs=1, space="PSUM") as scps,
            tc.tile_pool(name="ctxps", bufs=1, space="PSUM") as ctxps,
        ):
            # ---- persistent products -----------------------------------
            xT = big.tile([128, ND * S], mmdt, name="xT")
            xT_v = xT[:].rearrange("p (d t) -> p d t", d=ND)
            kv_latT = big.tile([128, S], mmdt, name="kv_latT")
            q_latT0 = big.tile([128, S], mmdt, name="q_latT0")
            q_latT1 = big.tile([128, S], mmdt, name="q_latT1")
            KT = big.tile([128, 4 * S], mmdt, name="KT")
            QT = big.tile([128, 4 * S], mmdt, name="QT")
            V = big.tile([128, NT * 520], mmdt, name="V")
            v_view = V[:].rearrange("p (k h c) -> p k h c", h=NHL, c=65)
            ctxT = big.tile([128, 4 * S], mmdt, name="ctxT")
            ctxT_v = ctxT[:].rearrange("p (c t) -> p c t", c=4)

            # ones columns of V (col 64 of each 65-wide head block)
            nc.gpsimd.memset(v_view[:, :, :, 64:65], 1.0)

            # ---- weights into SBUF (staged fp32 DMA, rounded to bf16) --
            def load_rounded(dst_ap, src_ap, shape):
                st = stg.tile([128, 1024], F32, tag="stage")
                sap = st[:shape[0], :shape[1]]
                nc.sync.dma_start(out=sap, in_=src_ap)
                nc.vector.tensor_copy(dst_ap, sap)

            w_kvc_sb = wts.tile([128, DIM], mmdt, name="w_kvc_sb")
            w_qc_sb = wts.tile([128, ND * QR], mmdt, name="w_qc_sb")
            w_kvu_k_sb = wts.tile([128, 512], mmdt, name="w_kvu_k_sb")
            w_kvu_v_sb = wts.tile([128, 512], mmdt, name="w_kvu_v_sb")
            w_qu_sb = wts.tile([128, 1024], mmdt, name="w_qu_sb")
            w_o_sb = wts.tile([128, 4 * DIM], mmdt, name="w_o_sb")
            b_all_sb = wts.tile([128, 11], F32, name="b_all_sb")
            b_kvc_sb = b_all_sb[:, 0:1]
            b_qc_sb = b_all_sb[:, 1:3]
            b_qu_sb = b_all_sb[:, 3:7]
            b_kvu_k_sb = b_all_sb[:, 7:11]

            def load_rounded3(dst_ap, src3_ap, nchunks, w):
                """One DMA of [128, nchunks, w] row-chunked DRAM weights."""
                st = stg.tile([128, 1024], F32, tag="stage")
                sap = st[:, :nchunks * w].rearrange("p (c q) -> p c q", c=nchunks)
                nc.sync.dma_start(out=sap, in_=src3_ap)
                nc.vector.tensor_copy(dst_ap, st[:, :nchunks * w])

            def wload_latent():
                nc.sync.dma_start(out=b_all_sb[:], in_=b_all_d[:, :])
                load_rounded3(w_kvc_sb[:],
                              w_kvc_d[:, :].rearrange("(c p) q -> p c q", p=128),
                              ND, 128)
                for g in range(2):
                    load_rounded3(
                        w_qc_sb[:, 1024 * g:1024 * g + 1024],
                        w_qc_d[512 * g:512 * g + 512, :].rearrange(
                            "(c p) q -> p c q", p=128),
                        4, QR)

            def wload_up():
                load_rounded3(w_qu_sb[:],
                              w_qu_d[:, :].rearrange("(c p) q -> p c q", p=128),
                              2, 512)
                load_rounded(w_kvu_k_sb[:], w_kvu_k_d[:, :], (128, 512))
                load_rounded(w_kvu_v_sb[:], w_kvu_v_d[:, :], (128, 512))


            def wload_o():
                for cc in range(4):
                    load_rounded(w_o_sb[:, DIM * cc:DIM * cc + DIM],
                                 w_o_d[128 * cc:128 * cc + 128, :], (128, DIM))

            # ---- emission helpers --------------------------------------
            def x_chunk(q):
                xf = xfp.tile([128, DIM], F32, tag="xf", bufs=2)
                nc.sync.dma_start(
                    out=xf[:], in_=x_d[128 * q:128 * q + 128, :])
                xb = xbp.tile([128, DIM], mmdt, tag="xb", bufs=3)
                nc.gpsimd.tensor_copy(xb[:], xf[:])
                nc.sync.dma_start_transpose(
                    xT_v[:, :, 128 * q:128 * q + 128], xb[:])

            def x_stage(p):
                """Load, round, and DMA-transpose x tokens [512p, 512p+512)."""
                for q in range(4 * p, 4 * p + 4):
                    xf = xfp.tile([128, DIM], F32, tag="xf", bufs=2)
                    nc.sync.dma_start(
                        out=xf[:], in_=x_d[128 * q:128 * q + 128, :])
                    xb = xbp.tile([128, DIM], mmdt, tag="xb", bufs=3)
                    nc.gpsimd.tensor_copy(xb[:], xf[:])
                    nc.sync.dma_start_transpose(
                        xT_v[:, :, 128 * q:128 * q + 128], xb[:])

            def piece(pj, p, to_bg=False):
                """All projections for tokens [512p, 512p+512).

                With to_bg=True the sub-steps are queued on `bg` and drained
                one per attention slot, so they fill engine-idle time instead
                of preempting the next unit's QK matmuls.
                """
                o = 512 * p
                items = []

                def _lat(w_sb, coloff, dst, b_ap):
                    def emit():
                        acc = pj.tile([128, 512], F32, tag="pj", bufs=2,
                                      name=f"pj_{p}_{coloff}")
                        for dc in range(ND):
                            nc.tensor.matmul(
                                acc[:], w_sb[:, QR * dc + coloff:QR * dc + coloff + 128]
                                if w_sb is w_qc_sb else
                                w_sb[:, 128 * dc:128 * dc + 128],
                                xT_v[:, dc, o:o + 512],
                                start=(dc == 0), stop=(dc == ND - 1))
                        nc.vector.tensor_scalar_add(dst[:, o:o + 512], acc[:], b_ap)
                    return emit

                items.append(_lat(w_kvc_sb, 0, kv_latT, b_kvc_sb))
                items.append(_lat(w_qc_sb, 0, q_latT0, b_qc_sb[:, 0:1]))
                items.append(_lat(w_qc_sb, 128, q_latT1, b_qc_sb[:, 1:2]))

                def _qt(c):
                    def emit():
                        qp2 = pj.tile([128, 512], F32, tag="pj", bufs=2,
                                      name=f"pjq_{p}_{c}")
                        nc.tensor.matmul(
                            qp2[:], w_qu_sb[:, 128 * c:128 * c + 128],
                            q_latT0[:, o:o + 512], start=True, stop=False)
                        nc.tensor.matmul(
                            qp2[:], w_qu_sb[:, 512 + 128 * c:512 + 128 * c + 128],
                            q_latT1[:, o:o + 512], start=False, stop=True)
                        nc.vector.tensor_scalar_add(
                            QT[:, c * S + o:c * S + o + 512], qp2[:],
                            b_qu_sb[:, c:c + 1])
                        kp = pj.tile([128, 512], F32, tag="pj", bufs=2,
                                     name=f"pjk_{p}_{c}")
                        nc.tensor.matmul(
                            kp[:], w_kvu_k_sb[:, 128 * c:128 * c + 128],
                            kv_latT[:, o:o + 512], start=True, stop=True)
                        nc.vector.tensor_scalar_add(
                            KT[:, c * S + o:c * S + o + 512], kp[:],
                            b_kvu_k_sb[:, c:c + 1])
                    return emit

                for c in range(4):
                    items.append(_qt(c))

                def _v(q):
                    def emit():
                        vp = pj.tile([128, 512], F32, tag="pj", bufs=2,
                                     name=f"pjv_{q}")
                        nc.tensor.matmul(vp[:], kv_latT[:, 128 * q:128 * q + 128],
                                         w_kvu_v_sb[:], start=True, stop=True)
                        nc.vector.tensor_copy(
                            v_view[:, q, :, 0:64],
                            vp[:].rearrange("p (h c) -> p h c", c=64))
                    return emit

                for q in range(4 * p, 4 * p + 4):
                    items.append(_v(q))
                if to_bg:
                    bg.extend(items)
                else:
                    for it in items:
                        it()

            pending = []  # deferred closures, drained into later QK/exp slots
            bg = []       # background closures (pieces, out_proj), 1 per slot

            def drain(n):
                for _ in range(min(n, len(pending))):
                    pending.pop(0)()

            def drain_bg(n):
                for _ in range(min(n, len(bg))):
                    bg.pop(0)()

            def attn_unit(hp, s0, SW=512, last=False):
                """Attention for queries [s0, s0+SW), both heads of pair hp.

                One merged exp per key-chunk covers both heads ([128, 2, fd]
                strided AP over a shared scores tile).  PV accumulation
                groups are emitted contiguously per (head, s-chunk) -- PSUM
                banks support only one open group at a time -- lagged into
                the QK/exp slots; trailing groups, the retires, and the
                ctxT transposes are deferred into the NEXT unit's slots via
                `pending` (the 4-deep PE wait queue would otherwise block
                the next unit's QK behind not-yet-ready PV matmuls).
                """
                base = s0 // 128
                nch = SW // 128
                kmax = base + nch
                cp = cpp.tile([128, 512], mmdt, tag="cp", name=f"cp_{hp}_{s0}")
                ctxs = [ctxps.tile([128, 512], F32, tag="ctx", bufs=2,
                                   name=f"ctx_{hp}_{s0}_{i}")
                        for i in range(2)]
                exs = []

                def pv_group(h2, c):
                    h = 2 * hp + h2
                    ctx = ctxs[h2]
                    klast = base + c

                    def emit():
                        for k in range(klast + 1):
                            rel = max(s0, 128 * k) - s0
                            cs = 512 * h2 + 128 * c - rel
                            nc.tensor.matmul(
                                ctx[:, 128 * c:128 * c + 65],
                                exs[k][:, cs:cs + 128],
                                V[:, 520 * k + 65 * h:520 * k + 65 * h + 65],
                                start=(k == 0), stop=(k == klast))
                        if last:
                            rec = attn.tile([128, 1], F32, tag="rec1", bufs=4,
                                            name=f"rec1_{hp}_{s0}_{h2}_{c}")
                            nc.vector.reciprocal(
                                rec[:], ctx[:, 128 * c + 64:128 * c + 65])
                            nc.vector.tensor_scalar_mul(
                                cp[:, 128 * c + 64 * h2:128 * c + 64 * h2 + 64],
                                ctx[:, 128 * c:128 * c + 64], rec[:, 0:1])
                            if h2 == 1:
                                nc.sync.dma_start_transpose(
                                    ctxT_v[:, hp, s0 + 128 * c:s0 + 128 * c + 128]
                                    .rearrange("p (b t) -> p b t", t=128),
                                    cp[:, 128 * c:128 * c + 128])
                    return emit

                def retire(h2):
                    ctx = ctxs[h2]

                    def emit():
                        rec = attn.tile([128, 4], F32, tag="rec", bufs=4,
                                        name=f"rec_{hp}_{s0}_{h2}")
                        nc.vector.reciprocal(
                            rec[:, :nch],
                            ctx[:].rearrange("p (c u) -> p c u", u=128)[:, :nch, 64])
                        for c in range(nch):
                            nc.vector.tensor_scalar_mul(
                                cp[:, 128 * c + 64 * h2:128 * c + 64 * h2 + 64],
                                ctx[:, 128 * c:128 * c + 64], rec[:, c:c + 1])
                    return emit

                def tp():
                    def emit():
                        nc.sync.dma_start_transpose(
                            ctxT_v[:, hp, s0:s0 + SW].rearrange(
                                "p (b t) -> p b t", t=128),
                            cp[:, :SW])
                    return emit

                lag = 2 if s0 == 0 else 0
                inslot = nch if last else max(1, nch - 2)
                emitted = 0
                for k in range(kmax):
                    t0 = 128 * k
                    ss = max(s0, t0)
                    fd = s0 + SW - ss
                    sc = scps.tile([128, 1024], F32, tag="sc", bufs=2,
                                   name=f"sc_{hp}_{s0}_{k}")
                    for h2 in range(2):
                        po = 64 * h2
                        nc.tensor.matmul(
                            sc[:, 512 * h2:512 * h2 + fd],
                            KT[po:po + 64, hp * S + t0:hp * S + t0 + 128],
                            QT[po:po + 64, hp * S + ss:hp * S + ss + fd],
                            start=True, stop=True)
                    ex = attn.tile([128, 1024], mmdt, tag="ex", bufs=21,
                                   name=f"ex_{hp}_{s0}_{k}")
                    exs.append(ex)
                    sc3 = sc[:].rearrange("p (g q) -> p g q", g=2)[:, :, :fd]
                    ex3 = ex[:].rearrange("p (g q) -> p g q", g=2)[:, :, :fd]
                    nc.scalar.activation(ex3, sc3, AF.Exp, scale=0.125)
                    if t0 >= s0:
                        nc.gpsimd.affine_select(
                            out=ex[:].rearrange("p (g q) -> p g q", g=2)[:, :, 0:128],
                            in_=ex[:].rearrange("p (g q) -> p g q", g=2)[:, :, 0:128],
                            pattern=[[0, 2], [1, 128]],
                            compare_op=mybir.AluOpType.is_ge,
                            fill=0.0, base=0, channel_multiplier=-1)
                    drain(2)
                    drain_bg(1)
                    c = k - base - lag
                    if 0 <= c < inslot:
                        pv_group(0, c)()
                        pv_group(1, c)()
                        emitted = c + 1
                for c in range(emitted, nch):
                    if last:
                        pv_group(0, c)()
                        pv_group(1, c)()
                    else:
                        pending.append(pv_group(0, c))
                        pending.append(pv_group(1, c))
                if not last:
                    pending.append(retire(0))
                    pending.append(retire(1))
                    pending.append(tp())

            def out_chunk(ops, si, to_bg=False):
                """out_proj for tokens [128si, 128si+128)."""
                ob = obp.tile([128, DIM], F32, tag="ob", name=f"ob_{si}")

                def _half(u):
                    def emit():
                        op = ops.tile([128, 512], F32, tag="pj", bufs=2,
                                      name=f"op_{si}_{u}")
                        for cc in range(4):
                            nc.tensor.matmul(
                                op[:],
                                ctxT_v[:, cc, 128 * si:128 * si + 128],
                                w_o_sb[:, DIM * cc + 512 * u:DIM * cc + 512 * u + 512],
                                start=(cc == 0), stop=(cc == 3))
                        nc.vector.tensor_copy(ob[:, 512 * u:512 * u + 512], op[:])
                        if u == 1:
                            nc.sync.dma_start(
                                out=out_d[128 * si:128 * si + 128, :], in_=ob[:])
                    return emit

                if to_bg:
                    bg.append(_half(0))
                    bg.append(_half(1))
                else:
                    _half(0)()
                    _half(1)()

            # ---- emission schedule -------------------------------------
            with tc.tile_pool(name="pjps", bufs=1, space="PSUM") as pj:
                x_stage(0)
                wload_latent()
                # interleave the second x piece with the up-projection weight
                # loads: w_qu gates piece-0's QT (the first attention pass),
                # the x chunks gate piece 1 (the second)
                load_rounded3(w_qu_sb[:],
                              w_qu_d[:, :].rearrange("(c p) q -> p c q", p=128),
                              2, 512)
                x_chunk(4)
                load_rounded(w_kvu_k_sb[:], w_kvu_k_d[:, :], (128, 512))
                x_chunk(5)
                load_rounded(w_kvu_v_sb[:], w_kvu_v_d[:, :], (128, 512))
                x_chunk(6)
                x_chunk(7)
                piece(pj, 0)
                for hp in range(4):
                    attn_unit(hp, 0)
                piece(pj, 1)
                x_stage(2)
                x_stage(3)
                # piece 2 fills s-block 1's idle (needed first by s-block 2);
                # piece 3 is deferred all the way to s-block 2 (needed by 3)
                piece(pj, 2, to_bg=True)
                for hp in range(4):
                    attn_unit(hp, 512)
                wload_o()
            with tc.tile_pool(name="ops", bufs=1, space="PSUM") as ops:
                piece(ops, 3, to_bg=True)
                for hp in range(4):
                    attn_unit(hp, 1024)
                    out_chunk(ops, hp, to_bg=True)
                for hp in range(4):
                    attn_unit(hp, 1536, last=(hp == 3))
                    if hp < 3:
                        out_chunk(ops, 4 + 2 * hp, to_bg=True)
                        out_chunk(ops, 5 + 2 * hp, to_bg=True)
                out_chunk(ops, 10, to_bg=True)
                out_chunk(ops, 11, to_bg=True)
                drain(len(pending))
                drain_bg(len(bg))
                for si in range(12, NT):
                    out_chunk(ops, si)

    nc.finalize()
    return nc


def shard_inputs(inputs, S=2048):
    """Build the 8 per-core input maps from full inputs."""
    f = lambda a: np.ascontiguousarray(np.asarray(a, dtype=np.float32))
    x = f(inputs["x"])
    w_kvc, b_kvc = f(inputs["w_kvc"]), f(inputs["b_kvc"])
    w_kvu, b_kvu = f(inputs["w_kvu"]), f(inputs["b_kvu"])
    w_qc, b_qc = f(inputs["w_qc"]), f(inputs["b_qc"])
    w_qu, b_qu = f(inputs["w_qu"]), f(inputs["b_qu"])
    w_o = f(inputs["w_o"])
    in_maps = []
    for core in range(NCORES):
        b = core // 2
        g = core % 2
        cs = slice(512 * g, 512 * g + 512)
        in_maps.append({
            "x": x[b],
            "w_kvc": w_kvc,
            "w_qc": w_qc,
            "w_kvu_k": np.ascontiguousarray(w_kvu[:, 512 * g:512 * g + 512]),
            "w_kvu_v": np.ascontiguousarray(w_kvu[:, 1024 + 512 * g:1024 + 512 * g + 512]),
            "w_qu": np.ascontiguousarray(w_qu[:, cs]),
            "w_o": np.ascontiguousarray(w_o[cs, :]),
            "b_all": np.ascontiguousarray(np.concatenate([
                b_kvc.reshape(128, 1),
                b_qc.reshape(2, 128).T,
                b_qu[cs].reshape(4, 128).T,
                b_kvu[cs].reshape(4, 128).T,
            ], axis=1)),
        })
    return in_maps


def gather_out(results, inputs, S=2048):
    """Sum the two per-batch partials and add the constant bias row."""
    f = lambda a: np.asarray(a, dtype=np.float32)
    b_v = f(inputs["b_kvu"])[DIM:]
    const_row = b_v @ f(inputs["w_o"]) + f(inputs["b_o"])
    out = np.empty((B, S, DIM), dtype=np.float32)
    for b in range(B):
        out[b] = results[2 * b]["out"] + results[2 * b + 1]["out"] + const_row
    return out


def kernel(**inputs) -> np.ndarray:
    from concourse.bass_utils import run_bass_kernel_spmd

    x = np.asarray(inputs["x"])
    S = x.shape[1]
    nc = build_mla(S=S)
    in_maps = shard_inputs(inputs, S=S)
    res = run_bass_kernel_spmd(nc, in_maps, list(range(NCORES))).results
    return gather_out(res, inputs, S=S)


# revision 33
# speedup vs baseline: 1.0444x; 1.0190x over previous
"""MLA (multi-head latent attention) Bass kernel for Trainium2, 8 NeuronCores.

Sharding: core i handles batch b = i // 2 and head-group g = i % 2
(8 of the 16 heads).  Each core computes a partial output (its heads'
contribution through out_proj); the host sums the two partials per batch
and adds a constant row (b_kvu_v @ w_o + b_o), which is exact because
softmax rows sum to 1 so the V-bias passes through attention additively.

All matmul operands are bf16 (1 cycle/row on the PE regardless of
output width); accumulation stays f32 in PSUM.  No PE transposes: both
x -> xT and ctx -> ctxT go through the DMA XBAR (dma_start_transpose,
2-byte dtypes) after an f32->bf16 rounding copy on GpSimd/DVE.

Pipeline (single TileContext; emission interleaved so attention starts
~20us in and out_proj overlaps the second attention half):
  piece(p), p=0..3 (512 tokens each):
    x chunks DMA'd, rounded to bf16 on Pool, DMA-transposed into
    xT [128, 8 d-chunks, S]; latents kv_latT [128,S], q_latT{0,1}
    [128,S] = W^T xT (+bias, DVE); KT/QT [128, 4 chunks * S] and
    V [128, NT*520] (64 cols/head + ones col for the softmax denom).
  attention(j, hp) per s-half j and head pair hp, heads sequential:
    scoresT [128 keys, 1024 queries] per key-chunk k via PE (64-row
    operands, disjoint groups per head); exp on ScalarE (scale=1/8,
    bf16 out); causal diagonal via affine_select on Pool; PV re-uses
    exp tiles as stationary: ctx_psum[s-chunk] [128 queries, 65]
    accumulates over k with the ones column giving the denominator.
    Retire: strided reciprocal [128,8] + 8 per-partition scalar
    multiplies (DVE) into a token-major bf16 pair tile, then one DMA
    transpose per (j,hp) into ctxT [128, 4 chunks * S].
  out_proj per 128-token chunk: 4x128-contraction accumulate into
  [128,512] PSUM halves, copies split DVE/Pool, DMA out.
"""

import numpy as np

import concourse.bass as bass
import concourse.bacc as bacc
import concourse.mybir as mybir
import concourse.tile as tile

DIM = 1024
NUM_HEADS = 16
HEAD_DIM = 64
LAT = 128
QR = 256
B = 4
NCORES = 8
ND = DIM // 128       # 8 d-chunks
NHL = 8               # heads per core
F32 = mybir.dt.float32
BF16 = mybir.dt.bfloat16
AF = mybir.ActivationFunctionType


def _pieces(total, w=512):
    return [(o, min(w, total - o)) for o in range(0, total, w)]


def build_mla(S=2048, mmdt=BF16):
    """Build the per-core Bass program (same SPMD program on all 8 cores)."""
    assert S % 512 == 0
    SH = S // 2           # s-half width
    NT = S // 128         # number of 128-token chunks
    NP = S // 512         # number of 512-token pieces

    nc = bacc.Bacc()

    x_d = nc.declare_dram_parameter("x", [S, DIM], F32, isOutput=False)
    w_kvc_d = nc.declare_dram_parameter("w_kvc", [DIM, LAT], F32, isOutput=False)
    w_qc_d = nc.declare_dram_parameter("w_qc", [DIM, QR], F32, isOutput=False)
    w_kvu_k_d = nc.declare_dram_parameter("w_kvu_k", [LAT, 512], F32, isOutput=False)
    w_kvu_v_d = nc.declare_dram_parameter("w_kvu_v", [LAT, 512], F32, isOutput=False)
    w_qu_d = nc.declare_dram_parameter("w_qu", [QR, 512], F32, isOutput=False)
    w_o_d = nc.declare_dram_parameter("w_o", [512, DIM], F32, isOutput=False)
    b_all_d = nc.declare_dram_parameter("b_all", [128, 11], F32, isOutput=False)
    out_d = nc.declare_dram_parameter("out", [S, DIM], F32, isOutput=True)

    with tile.TileContext(nc) as tc:
        with (
            tc.tile_pool(name="wts", bufs=1) as wts,
            tc.tile_pool(name="big", bufs=1) as big,
            tc.tile_pool(name="stg", bufs=2) as stg,
            tc.tile_pool(name="xfp", bufs=2) as xfp,
            tc.tile_pool(name="xbp", bufs=2) as xbp,
            tc.tile_pool(name="attn", bufs=1) as attn,
            tc.tile_pool(name="cpp", bufs=2) as cpp,
            tc.tile_pool(name="obp", bufs=3) as obp,
            tc.tile_pool(name="scps", bufs=1, space="PSUM") as scps,
            tc.tile_pool(name="ctxps", bufs=1, space="PSUM") as ctxps,
        ):
            # ---- persistent products -----------------------------------
            xT = big.tile([128, ND * S], mmdt, name="xT")
            xT_v = xT[:].rearrange("p (d t) -> p d t", d=ND)
            kv_latT = big.tile([128, S], mmdt, name="kv_latT")
            q_latT0 = big.tile([128, S], mmdt, name="q_latT0")
            q_latT1 = big.tile([128, S], mmdt, name="q_latT1")
            KT = big.tile([128, 4 * S], mmdt, name="KT")
            QT = big.tile([128, 4 * S], mmdt, name="QT")
            V = big.tile([128, NT * 520], mmdt, name="V")
            v_view = V[:].rearrange("p (k h c) -> p k h c", h=NHL, c=65)
            ctxT = big.tile([128, 4 * S], mmdt, name="ctxT")
            ctxT_v = ctxT[:].rearrange("p (c t) -> p c t", c=4)

            # ones columns of V (col 64 of each 65-wide head block)
            nc.gpsimd.memset(v_view[:, :, :, 64:65], 1.0)

            # ---- weights into SBUF (staged fp32 DMA, rounded to bf16) --
            def load_rounded(dst_ap, src_ap, shape):
                st = stg.tile([128, 1024], F32, tag="stage")
                sap = st[:shape[0], :shape[1]]
                nc.sync.dma_start(out=sap, in_=src_ap)
                nc.vector.tensor_copy(dst_ap, sap)

            w_kvc_sb = wts.tile([128, DIM], mmdt, name="w_kvc_sb")
            w_qc_sb = wts.tile([128, ND * QR], mmdt, name="w_qc_sb")
            w_kvu_k_sb = wts.tile([128, 512], mmdt, name="w_kvu_k_sb")
            w_kvu_v_sb = wts.tile([128, 512], mmdt, name="w_kvu_v_sb")
            w_qu_sb = wts.tile([128, 1024], mmdt, name="w_qu_sb")
            w_o_sb = wts.tile([128, 4 * DIM], mmdt, name="w_o_sb")
            b_all_sb = wts.tile([128, 11], F32, name="b_all_sb")
            b_kvc_sb = b_all_sb[:, 0:1]
            b_qc_sb = b_all_sb[:, 1:3]
            b_qu_sb = b_all_sb[:, 3:7]
            b_kvu_k_sb = b_all_sb[:, 7:11]

            def load_rounded3(dst_ap, src3_ap, nchunks, w):
                """One DMA of [128, nchunks, w] row-chunked DRAM weights."""
                st = stg.tile([128, 1024], F32, tag="stage")
                sap = st[:, :nchunks * w].rearrange("p (c q) -> p c q", c=nchunks)
                nc.sync.dma_start(out=sap, in_=src3_ap)
                nc.vector.tensor_copy(dst_ap, st[:, :nchunks * w])

            def wload_up():
                load_rounded3(w_qu_sb[:],
                              w_qu_d[:, :].rearrange("(c p) q -> p c q", p=128),
                              2, 512)
                load_rounded(w_kvu_k_sb[:], w_kvu_k_d[:, :], (128, 512))
                load_rounded(w_kvu_v_sb[:], w_kvu_v_d[:, :], (128, 512))


            def wload_o():
                for cc in range(4):
                    load_rounded(w_o_sb[:, DIM * cc:DIM * cc + DIM],
                                 w_o_d[128 * cc:128 * cc + 128, :], (128, DIM))

            # ---- emission helpers --------------------------------------
            def x_chunk(q):
                xf = xfp.tile([128, DIM], F32, tag="xf", bufs=3)
                nc.sync.dma_start(
                    out=xf[:], in_=x_d[128 * q:128 * q + 128, :])
                xb = xbp.tile([128, DIM], mmdt, tag="xb", bufs=2)
                nc.gpsimd.tensor_copy(xb[:], xf[:])
                nc.sync.dma_start_transpose(
                    xT_v[:, :, 128 * q:128 * q + 128], xb[:])

            def x_stage(p):
                """Load, round, and DMA-transpose x tokens [512p, 512p+512)."""
                for q in range(4 * p, 4 * p + 4):
                    xf = xfp.tile([128, DIM], F32, tag="xf", bufs=3)
                    nc.sync.dma_start(
                        out=xf[:], in_=x_d[128 * q:128 * q + 128, :])
                    xb = xbp.tile([128, DIM], mmdt, tag="xb", bufs=2)
                    nc.gpsimd.tensor_copy(xb[:], xf[:])
                    nc.sync.dma_start_transpose(
                        xT_v[:, :, 128 * q:128 * q + 128], xb[:])

            def piece(pj, p, to_bg=False):
                """All projections for tokens [512p, 512p+512).

                With to_bg=True the sub-steps are queued on `bg` and drained
                one per attention slot, so they fill engine-idle time instead
                of preempting the next unit's QK matmuls.
                """
                o = 512 * p
                items = []

                def _lat(w_sb, is_qc, coloff, dst, b_ap):
                    state = {}

                    def ap_for(dc):
                        if is_qc:
                            return w_sb[:, QR * dc + coloff:QR * dc + coloff + 128]
                        return w_sb[:, 128 * dc:128 * dc + 128]

                    def part(d0, d1):
                        def emit():
                            if 'acc' not in state:
                                state['acc'] = pj.tile(
                                    [128, 512], F32, tag="pj", bufs=2,
                                    name=f"pj_{p}_{coloff}_{id(state) % 97}")
                            acc = state['acc']
                            for dc in range(d0, d1):
                                nc.tensor.matmul(
                                    acc[:], ap_for(dc),
                                    xT_v[:, dc, o:o + 512],
                                    start=(dc == 0), stop=(dc == ND - 1))
                            if d1 == ND:
                                nc.vector.tensor_scalar_add(
                                    dst[:, o:o + 512], acc[:], b_ap)
                        return emit
                    return [part(0, 3), part(3, 6), part(6, 8)]

                items.extend(_lat(w_kvc_sb, False, 0, kv_latT, b_kvc_sb))
                items.extend(_lat(w_qc_sb, True, 0, q_latT0, b_qc_sb[:, 0:1]))
                items.extend(_lat(w_qc_sb, True, 128, q_latT1, b_qc_sb[:, 1:2]))

                def _qt(c):
                    def emit():
                        qp2 = pj.tile([128, 512], F32, tag="pj", bufs=2,
                                      name=f"pjq_{p}_{c}")
                        nc.tensor.matmul(
                            qp2[:], w_qu_sb[:, 128 * c:128 * c + 128],
                            q_latT0[:, o:o + 512], start=True, stop=False)
                        nc.tensor.matmul(
                            qp2[:], w_qu_sb[:, 512 + 128 * c:512 + 128 * c + 128],
                            q_latT1[:, o:o + 512], start=False, stop=True)
                        nc.vector.tensor_scalar_add(
                            QT[:, c * S + o:c * S + o + 512], qp2[:],
                            b_qu_sb[:, c:c + 1])
                    return emit

                def _kt(c):
                    def emit():
                        kp = pj.tile([128, 512], F32, tag="pj", bufs=2,
                                     name=f"pjk_{p}_{c}")
                        nc.tensor.matmul(
                            kp[:], w_kvu_k_sb[:, 128 * c:128 * c + 128],
                            kv_latT[:, o:o + 512], start=True, stop=True)
                        nc.vector.tensor_scalar_add(
                            KT[:, c * S + o:c * S + o + 512], kp[:],
                            b_kvu_k_sb[:, c:c + 1])
                    return emit

                for c in range(4):
                    items.append(_qt(c))
                    items.append(_kt(c))

                def _v(q):
                    def emit():
                        vp = pj.tile([128, 512], F32, tag="pj", bufs=2,
                                     name=f"pjv_{q}")
                        nc.tensor.matmul(vp[:], kv_latT[:, 128 * q:128 * q + 128],
                                         w_kvu_v_sb[:], start=True, stop=True)
                        nc.vector.tensor_copy(
                            v_view[:, q, :, 0:64],
                            vp[:].rearrange("p (h c) -> p h c", c=64))
                    return emit

                for q in range(4 * p, 4 * p + 4):
                    items.append(_v(q))
                if to_bg:
                    bg.extend(items)
                else:
                    for it in items:
                        it()

            pending = []  # deferred closures, drained into later QK/exp slots
            bg = []       # background closures (pieces, out_proj), 1 per slot

            def drain(n):
                for _ in range(min(n, len(pending))):
                    pending.pop(0)()

            def drain_bg(n):
                for _ in range(min(n, len(bg))):
                    bg.pop(0)()

            def attn_unit(hp, s0, SW=512, last=False):
                """Attention for queries [s0, s0+SW), both heads of pair hp.

                One merged exp per key-chunk covers both heads ([128, 2, fd]
                strided AP over a shared scores tile).  PV accumulation
                groups are emitted contiguously per (head, s-chunk) -- PSUM
                banks support only one open group at a time -- lagged into
                the QK/exp slots; trailing groups, the retires, and the
                ctxT transposes are deferred into the NEXT unit's slots via
                `pending` (the 4-deep PE wait queue would otherwise block
                the next unit's QK behind not-yet-ready PV matmuls).
                """
                base = s0 // 128
                nch = SW // 128
                kmax = base + nch
                cp = cpp.tile([128, 512], mmdt, tag="cp", name=f"cp_{hp}_{s0}")
                ctxs = [ctxps.tile([128, 512], F32, tag="ctx", bufs=2,
                                   name=f"ctx_{hp}_{s0}_{i}")
                        for i in range(2)]
                exs = []

                def pv_group(h2, c):
                    h = 2 * hp + h2
                    ctx = ctxs[h2]
                    klast = base + c

                    def emit():
                        for k in range(klast + 1):
                            rel = max(s0, 128 * k) - s0
                            cs = 512 * h2 + 128 * c - rel
                            nc.tensor.matmul(
                                ctx[:, 128 * c:128 * c + 65],
                                exs[k][:, cs:cs + 128],
                                V[:, 520 * k + 65 * h:520 * k + 65 * h + 65],
                                start=(k == 0), stop=(k == klast))
                        if last:
                            rec = attn.tile([128, 1], F32, tag="rec1", bufs=4,
                                            name=f"rec1_{hp}_{s0}_{h2}_{c}")
                            nc.vector.reciprocal(
                                rec[:], ctx[:, 128 * c + 64:128 * c + 65])
                            nc.vector.tensor_scalar_mul(
                                cp[:, 128 * c + 64 * h2:128 * c + 64 * h2 + 64],
                                ctx[:, 128 * c:128 * c + 64], rec[:, 0:1])
                            if h2 == 1:
                                nc.sync.dma_start_transpose(
                                    ctxT_v[:, hp, s0 + 128 * c:s0 + 128 * c + 128]
                                    .rearrange("p (b t) -> p b t", t=128),
                                    cp[:, 128 * c:128 * c + 128])
                    return emit

                def retire(h2):
                    ctx = ctxs[h2]

                    def emit():
                        rec = attn.tile([128, 4], F32, tag="rec", bufs=4,
                                        name=f"rec_{hp}_{s0}_{h2}")
                        nc.vector.reciprocal(
                            rec[:, :nch],
                            ctx[:].rearrange("p (c u) -> p c u", u=128)[:, :nch, 64])
                        for c in range(nch):
                            nc.vector.tensor_scalar_mul(
                                cp[:, 128 * c + 64 * h2:128 * c + 64 * h2 + 64],
                                ctx[:, 128 * c:128 * c + 64], rec[:, c:c + 1])
                    return emit

                def tp():
                    def emit():
                        nc.sync.dma_start_transpose(
                            ctxT_v[:, hp, s0:s0 + SW].rearrange(
                                "p (b t) -> p b t", t=128),
                            cp[:, :SW])
                    return emit

                lag = 2 if s0 == 0 else 0
                inslot = nch if last else max(1, nch - 2)
                emitted = 0
                for k in range(kmax):
                    t0 = 128 * k
                    ss = max(s0, t0)
                    fd = s0 + SW - ss
                    sc = scps.tile([128, 1024], F32, tag="sc", bufs=2,
                                   name=f"sc_{hp}_{s0}_{k}")
                    for h2 in range(2):
                        po = 64 * h2
                        nc.tensor.matmul(
                            sc[:, 512 * h2:512 * h2 + fd],
                            KT[po:po + 64, hp * S + t0:hp * S + t0 + 128],
                            QT[po:po + 64, hp * S + ss:hp * S + ss + fd],
                            start=True, stop=True)
                    ex = attn.tile([128, 1024], mmdt, tag="ex", bufs=21,
                                   name=f"ex_{hp}_{s0}_{k}")
                    exs.append(ex)
                    sc3 = sc[:].rearrange("p (g q) -> p g q", g=2)[:, :, :fd]
                    ex3 = ex[:].rearrange("p (g q) -> p g q", g=2)[:, :, :fd]
                    nc.scalar.activation(ex3, sc3, AF.Exp, scale=0.125)
                    if t0 >= s0:
                        nc.gpsimd.affine_select(
                            out=ex[:].rearrange("p (g q) -> p g q", g=2)[:, :, 0:128],
                            in_=ex[:].rearrange("p (g q) -> p g q", g=2)[:, :, 0:128],
                            pattern=[[0, 2], [1, 128]],
                            compare_op=mybir.AluOpType.is_ge,
                            fill=0.0, base=0, channel_multiplier=-1)
                    drain(2)
                    drain_bg(1)
                    c = k - base - lag
                    if 0 <= c < inslot:
                        pv_group(0, c)()
                        pv_group(1, c)()
                        emitted = c + 1
                for c in range(emitted, nch):
                    if last:
                        pv_group(0, c)()
                        pv_group(1, c)()
                    else:
                        pending.append(pv_group(0, c))
                        pending.append(pv_group(1, c))
                if not last:
                    pending.append(retire(0))
                    pending.append(retire(1))
                    pending.append(tp())

            def out_chunk(ops, si, to_bg=False):
                """out_proj for tokens [128si, 128si+128)."""
                ob = obp.tile([128, DIM], F32, tag="ob", name=f"ob_{si}")

                def _half(u):
                    def emit():
                        op = ops.tile([128, 512], F32, tag="pj", bufs=2,
                                      name=f"op_{si}_{u}")
                        for cc in range(4):
                            nc.tensor.matmul(
                                op[:],
                                ctxT_v[:, cc, 128 * si:128 * si + 128],
                                w_o_sb[:, DIM * cc + 512 * u:DIM * cc + 512 * u + 512],
                                start=(cc == 0), stop=(cc == 3))
                        nc.vector.tensor_copy(ob[:, 512 * u:512 * u + 512], op[:])
                        if u == 1:
                            nc.sync.dma_start(
                                out=out_d[128 * si:128 * si + 128, :], in_=ob[:])
                    return emit

                if to_bg:
                    bg.append(_half(0))
                    bg.append(_half(1))
                else:
                    _half(0)()
                    _half(1)()

            # ---- emission schedule -------------------------------------
            with tc.tile_pool(name="pjps", bufs=1, space="PSUM") as pj:
                x_stage(0)
                load_rounded3(w_kvc_sb[:],
                              w_kvc_d[:, :].rearrange("(c p) q -> p c q", p=128),
                              ND, 128)
                load_rounded3(w_qu_sb[:],
                              w_qu_d[:, :].rearrange("(c p) q -> p c q", p=128),
                              2, 512)
                for g in range(2):
                    load_rounded3(
                        w_qc_sb[:, 1024 * g:1024 * g + 1024],
                        w_qc_d[512 * g:512 * g + 512, :].rearrange(
                            "(c p) q -> p c q", p=128),
                        4, QR)
                nc.sync.dma_start(out=b_all_sb[:], in_=b_all_d[:, :])
                load_rounded(w_kvu_k_sb[:], w_kvu_k_d[:, :], (128, 512))
                x_chunk(4)
                load_rounded(w_kvu_v_sb[:], w_kvu_v_d[:, :], (128, 512))
                x_chunk(5)
                x_chunk(6)
                x_chunk(7)
                piece(pj, 0)
                for hp in range(4):
                    attn_unit(hp, 0)
                piece(pj, 1)
                x_stage(2)
                x_stage(3)
                # piece 2 fills s-block 1's idle (needed first by s-block 2);
                # piece 3 is deferred all the way to s-block 2 (needed by 3)
                piece(pj, 2, to_bg=True)
                for hp in range(4):
                    attn_unit(hp, 512)
                wload_o()
            with tc.tile_pool(name="ops", bufs=1, space="PSUM") as ops:
                piece(ops, 3, to_bg=True)
                for hp in range(4):
                    attn_unit(hp, 1024)
                    out_chunk(ops, hp, to_bg=True)
                for hp in range(4):
                    attn_unit(hp, 1536, last=(hp == 3))
                    if hp < 3:
                        out_chunk(ops, 4 + 2 * hp, to_bg=True)
                        out_chunk(ops, 5 + 2 * hp, to_bg=True)
                out_chunk(ops, 10, to_bg=True)
                out_chunk(ops, 11, to_bg=True)
                drain(len(pending))
                drain_bg(len(bg))
                for si in range(12, NT):
                    out_chunk(ops, si)

    nc.finalize()
    return nc


def shard_inputs(inputs, S=2048):
    """Build the 8 per-core input maps from full inputs."""
    f = lambda a: np.ascontiguousarray(np.asarray(a, dtype=np.float32))
    x = f(inputs["x"])
    w_kvc, b_kvc = f(inputs["w_kvc"]), f(inputs["b_kvc"])
    w_kvu, b_kvu = f(inputs["w_kvu"]), f(inputs["b_kvu"])
    w_qc, b_qc = f(inputs["w_qc"]), f(inputs["b_qc"])
    w_qu, b_qu = f(inputs["w_qu"]), f(inputs["b_qu"])
    w_o = f(inputs["w_o"])
    in_maps = []
    for core in range(NCORES):
        b = core // 2
        g = core % 2
        cs = slice(512 * g, 512 * g + 512)
        in_maps.append({
            "x": x[b],
            "w_kvc": w_kvc,
            "w_qc": w_qc,
            "w_kvu_k": np.ascontiguousarray(w_kvu[:, 512 * g:512 * g + 512]),
            "w_kvu_v": np.ascontiguousarray(w_kvu[:, 1024 + 512 * g:1024 + 512 * g + 512]),
            "w_qu": np.ascontiguousarray(w_qu[:, cs]),
            "w_o": np.ascontiguousarray(w_o[cs, :]),
            "b_all": np.ascontiguousarray(np.concatenate([
                b_kvc.reshape(128, 1),
                b_qc.reshape(2, 128).T,
                b_qu[cs].reshape(4, 128).T,
                b_kvu[cs].reshape(4, 128).T,
            ], axis=1)),
        })
    return in_maps


def gather_out(results, inputs, S=2048):
    """Sum the two per-batch partials and add the constant bias row."""
    f = lambda a: np.asarray(a, dtype=np.float32)
    b_v = f(inputs["b_kvu"])[DIM:]
    const_row = b_v @ f(inputs["w_o"]) + f(inputs["b_o"])
    out = np.empty((B, S, DIM), dtype=np.float32)
    for b in range(B):
        out[b] = results[2 * b]["out"] + results[2 * b + 1]["out"] + const_row
    return out


def kernel(**inputs) -> np.ndarray:
    from concourse.bass_utils import run_bass_kernel_spmd

    x = np.asarray(inputs["x"])
    S = x.shape[1]
    nc = build_mla(S=S)
    in_maps = shard_inputs(inputs, S=S)
    res = run_bass_kernel_spmd(nc, in_maps, list(range(NCORES))).results
    return gather_out(res, inputs, S=S)
